# revision 1
# baseline (speedup 1.0000x reference)
import numpy as np

import concourse.bacc as bacc
import concourse.bass as bass
import concourse.tile as tile
from concourse import mybir
from concourse.bass_utils import run_bass_kernel_spmd

F32 = mybir.dt.float32
F32R = mybir.dt.float32r
RELU = mybir.ActivationFunctionType.Relu

N_CORES = 8
B_FULL = 65536
D = 768
NCHUNK = 6  # 768 / 128
PACK_W = 1454  # packed weight columns (see prep_weights)


def build_program(per_rows: int, chain_dt=F32R, pw=0.5, sw=0.25,
                  finalize=True, passes=1, tr2=False, accdma=False) -> bass.Bass:
    """One core's program: x [nb, 128, 4, 768] -> out [2, per_rows] (transposed).

    passes>1 repeats the whole pipeline (benchmarking only): pass p writes
    out columns [p*per_rows, (p+1)*per_rows)."""
    assert per_rows % 512 == 0
    nb = per_rows // 128  # b-tiles
    nst = nb // 4  # super-tiles of 512 rows

    # Bacc (not Bass): finalize() runs move_matmul_waits_to_ldweights +
    # generate_event_semaphores, without which walrus rejects multi-wait
    # Matmults ("Too many sync wait commands")
    nc = bacc.Bacc()
    x_ext = nc.dram_tensor("x", [nb, 128, 4, D], F32, kind="ExternalInput")
    # all weights + identity packed into one input (fewer per-exec binds);
    # column map: see prep_weights PACK_COLS
    wp_ext = nc.dram_tensor("wpack", [128, PACK_W], chain_dt,
                            kind="ExternalInput")
    out_ext = nc.dram_tensor("out", [2, passes * per_rows], F32, kind="ExternalOutput")

    with tile.TileContext(nc) as tc:
        with (
            tc.tile_pool(name="const", bufs=1) as cpool,
            tc.tile_pool(name="x", bufs=3) as xpool,
            tc.tile_pool(name="uv", bufs=3) as uvpool,
            tc.tile_pool(name="stage", bufs=2) as stpool,
            tc.tile_pool(name="chain_sb", bufs=2) as csb,
            tc.tile_pool(name="smalls", bufs=8) as smpool,
            tc.tile_pool(name="tpsum", bufs=2, space=bass.MemorySpace.PSUM) as tpsum,
            tc.tile_pool(name="cpsum", bufs=2, space=bass.MemorySpace.PSUM) as cpsum,
        ):
            # --- constants ---
            w1s_t = cpool.tile([128, NCHUNK, 96], chain_dt)
            w1e_t = cpool.tile([128, NCHUNK, 96], chain_dt)
            for c in range(NCHUNK):
                nc.sync.dma_start(w1s_t[:, c, :],
                                  wp_ext[:, c * 96:(c + 1) * 96])
                nc.sync.dma_start(w1e_t[:, c, :],
                                  wp_ext[:, 576 + c * 96:576 + (c + 1) * 96])
            w2s_t = cpool.tile([96, 48], chain_dt)
            w2e_t = cpool.tile([96, 48], chain_dt)
            w3s_t = cpool.tile([48, 24], chain_dt)
            w3e_t = cpool.tile([48, 24], chain_dt)
            w4s_t = cpool.tile([24, 12], chain_dt)
            w4e_t = cpool.tile([24, 12], chain_dt)
            w5sp_t = cpool.tile([12, 2], chain_dt)
            w5ss_t = cpool.tile([12, 2], chain_dt)
            w5e2_t = cpool.tile([12, 2], chain_dt)
            idn_t = cpool.tile([128, 128], chain_dt)
            out_sb = cpool.tile([2, per_rows], F32)
            for t, r, c0, w in [
                (w2s_t[:], 96, 1152, 48), (w2e_t[:], 96, 1200, 48),
                (w3s_t[:], 48, 1248, 24), (w3e_t[:], 48, 1272, 24),
                (w4s_t[:], 24, 1296, 12), (w4e_t[:], 24, 1308, 12),
                (w5sp_t[:], 12, 1320, 2), (w5ss_t[:], 12, 1322, 2),
                (w5e2_t[:], 12, 1324, 2), (idn_t[:], 128, 1326, 128),
            ]:
                nc.sync.dma_start(t, wp_ext[0:r, c0:c0 + w])

            stages = {}

            def emit_btile_group(ps, st):
                # stage layout: [128 feat_part, 4 bt, 6 chunk, 128 row]
                stage_pair = stpool.tile([128, 4, NCHUNK, 128], chain_dt)
                stage_seq = stpool.tile([128, 4, NCHUNK, 128], chain_dt)
                stages[(ps, st)] = (stage_pair, stage_seq)
                for bt4 in range(4):
                    bt = st * 4 + bt4
                    if accdma:
                        # uv = (x0,x1) + (x2,x3) computed by the SDMA CCE
                        # units: plain DMA of slots 0:2, accumulate 2:4.
                        uv0 = uvpool.tile([128, 3 if tr2 else 2, D], chain_dt)
                        nc.gpsimd.dma_start(uv0[:, 0:2, :],
                                            x_ext[bt][:, 0:2, :])
                        nc.gpsimd.dma_start(uv0[:, 0:2, :],
                                            x_ext[bt][:, 2:4, :],
                                            accum_op=mybir.AluOpType.add)
                        if tr2:
                            nc.vector.tensor_add(uv0[:, 2, :], uv0[:, 0, :],
                                                 uv0[:, 1, :])
                            tp = tpsum.tile([128, 2, NCHUNK, 128], chain_dt)
                            for c in range(NCHUNK):
                                u_c = uv0[:, 0, c * 128:(c + 1) * 128]
                                w_c = uv0[:, 2, c * 128:(c + 1) * 128]
                                nc.tensor.matmul(tp[:, 0, c, :], u_c, idn_t[:],
                                                 is_transpose=True, start=True,
                                                 stop=True)
                                nc.tensor.matmul(tp[:, 1, c, :], w_c, idn_t[:],
                                                 is_transpose=True, start=True,
                                                 stop=True)
                        else:
                            tp = tpsum.tile([128, 2, NCHUNK, 128], chain_dt)
                            for c in range(NCHUNK):
                                u_c = uv0[:, 0, c * 128:(c + 1) * 128]
                                v_c = uv0[:, 1, c * 128:(c + 1) * 128]
                                nc.tensor.matmul(tp[:, 0, c, :], u_c, idn_t[:],
                                                 is_transpose=True, start=True,
                                                 stop=True)
                                nc.tensor.matmul(tp[:, 1, c, :], u_c, idn_t[:],
                                                 is_transpose=True, start=True,
                                                 stop=False)
                                nc.tensor.matmul(tp[:, 1, c, :], v_c, idn_t[:],
                                                 is_transpose=True, start=False,
                                                 stop=True)
                        nc.scalar.activation(stage_pair[:, bt4], tp[:, 0],
                                             RELU, scale=pw)
                        nc.scalar.activation(stage_seq[:, bt4], tp[:, 1],
                                             RELU, scale=sw)
                        continue
                    xt = xpool.tile([128, 4, D], F32)
                    nc.gpsimd.dma_start(xt[:], x_ext[bt])
                    if tr2:
                        # u=x0+x2, v=x1+x3, w=u+v; transpose u and w only
                        uvw = uvpool.tile([128, 3, D], chain_dt)
                        nc.vector.tensor_add(uvw[:, 0:2, :], xt[:, 0:2, :],
                                             xt[:, 2:4, :])
                        nc.vector.tensor_add(uvw[:, 2, :], uvw[:, 0, :],
                                             uvw[:, 1, :])
                        tp = tpsum.tile([128, 2, NCHUNK, 128], chain_dt)
                        for c in range(NCHUNK):
                            u_c = uvw[:, 0, c * 128:(c + 1) * 128]
                            w_c = uvw[:, 2, c * 128:(c + 1) * 128]
                            nc.tensor.matmul(tp[:, 0, c, :], u_c, idn_t[:],
                                             is_transpose=True, start=True,
                                             stop=True)
                            nc.tensor.matmul(tp[:, 1, c, :], w_c, idn_t[:],
                                             is_transpose=True, start=True,
                                             stop=True)
                    else:
                        uv = uvpool.tile([128, 2, D], chain_dt)
                        # (u, v) = (x0, x1) + (x2, x3)
                        nc.vector.tensor_add(uv[:], xt[:, 0:2, :], xt[:, 2:4, :])
                        tp = tpsum.tile([128, 2, NCHUNK, 128], chain_dt)
                        for c in range(NCHUNK):
                            u_c = uv[:, 0, c * 128:(c + 1) * 128]
                            v_c = uv[:, 1, c * 128:(c + 1) * 128]
                            # pairT = u^T ; seqT = u^T + v^T (scales in drain)
                            nc.tensor.matmul(tp[:, 0, c, :], u_c, idn_t[:],
                                             is_transpose=True, start=True,
                                             stop=True)
                            nc.tensor.matmul(tp[:, 1, c, :], u_c, idn_t[:],
                                             is_transpose=True, start=True,
                                             stop=False)
                            nc.tensor.matmul(tp[:, 1, c, :], v_c, idn_t[:],
                                             is_transpose=True, start=False,
                                             stop=True)
                    # scaled relu drains PSUM -> stage (ACT)
                    nc.scalar.activation(stage_pair[:, bt4], tp[:, 0], RELU,
                                         scale=pw)
                    nc.scalar.activation(stage_seq[:, bt4], tp[:, 1], RELU,
                                         scale=sw)

            def emit_chains(ps, st):
                stage_pair, stage_seq = stages.pop((ps, st))
                # L1: 4 chains (pair_s, pair_e, seq_s, seq_e)
                l1_sb = []
                for stg, w1 in [(stage_pair, w1s_t), (stage_pair, w1e_t),
                                (stage_seq, w1s_t), (stage_seq, w1e_t)]:
                    l1 = cpsum.tile([96, 512], F32, tag="c")
                    for c in range(NCHUNK):
                        nc.tensor.matmul(l1[:], w1[:, c, :],
                                         stg[:, :, c, :],
                                         start=(c == 0), stop=(c == NCHUNK - 1))
                    sb = csb.tile([96, 512], chain_dt, tag="l1sb", bufs=6)
                    nc.scalar.activation(sb[:], l1[:], RELU)
                    l1_sb.append(sb)
                # chains in order: pair_s, pair_e, seq_s, seq_e — every matmul
                # PSUM dst at partition 0 (walrus anchors col_grp at column 0)
                brs = ["s", "e", "s", "e"]
                w2 = {"s": w2s_t, "e": w2e_t}
                w3 = {"s": w3s_t, "e": w3e_t}
                w4 = {"s": w4s_t, "e": w4e_t}
                l2_sb = []
                for br, sb1 in zip(brs, l1_sb):
                    l2 = cpsum.tile([48, 512], F32, tag="c")
                    nc.tensor.matmul(l2[:], w2[br][:], sb1[:],
                                     start=True, stop=True)
                    sb = csb.tile([48, 512], chain_dt, tag="l2sb", bufs=4)
                    nc.vector.tensor_scalar_max(sb[:], l2[:], 0.0)
                    l2_sb.append(sb)
                l3_sb = []
                for br, sb2 in zip(brs, l2_sb):
                    l3 = cpsum.tile([24, 512], F32, tag="c")
                    nc.tensor.matmul(l3[:], w3[br][:], sb2[:],
                                     start=True, stop=True)
                    sb = csb.tile([24, 512], chain_dt, tag="l3sb", bufs=4)
                    nc.scalar.activation(sb[:], l3[:], RELU)
                    l3_sb.append(sb)
                l4_sb = []
                for br, sb3 in zip(brs, l3_sb):
                    l4 = cpsum.tile([12, 512], F32, tag="c")
                    nc.tensor.matmul(l4[:], w4[br][:], sb3[:],
                                     start=True, stop=True)
                    sb = csb.tile([12, 512], chain_dt, tag="l4sb", bufs=4)
                    nc.scalar.activation(sb[:], l4[:], RELU)
                    l4_sb.append(sb)
                # L5: ps@w5sp, pe@w5e2, ss@w5ss, se@w5e2
                l5_sb = []
                for w5, sb4 in [(w5sp_t, l4_sb[0]), (w5e2_t, l4_sb[1]),
                                (w5ss_t, l4_sb[2]), (w5e2_t, l4_sb[3])]:
                    p = cpsum.tile([2, 512], F32, tag="c")
                    nc.tensor.matmul(p[:], w5[:], sb4[:], start=True, stop=True)
                    sb = smpool.tile([2, 512], F32, tag="l5sb", bufs=6)
                    nc.vector.tensor_copy(sb[:], p[:])
                    l5_sb.append(sb)
                sp, ep, ss, es = l5_sb
                # cross + final: out = s_pair*esum_pair + s_seq*esum_seq
                t1 = smpool.tile([2, 512], F32, tag="t1", bufs=2)
                t2 = smpool.tile([2, 512], F32, tag="t2", bufs=2)
                nc.vector.tensor_mul(t1[:], sp[:], ep[:])
                nc.vector.tensor_mul(t2[:], ss[:], es[:])
                col = st * 512
                nc.vector.tensor_add(out_sb[:, col:col + 512], t1[:], t2[:])

            # 1-super-tile software pipeline so each engine's in-order queue
            # never waits on a later-emitted producer
            for ps in range(passes):
                for st in range(nst + 1):
                    if st < nst:
                        emit_btile_group(ps, st)
                    if st >= 1:
                        emit_chains(ps, st - 1)
                nc.sync.dma_start(
                    out_ext[:, ps * per_rows:(ps + 1) * per_rows], out_sb[:])

    if finalize:
        nc.finalize()
    return nc


def prep_weights(sW1, sW2, sW3, sW4, sW5, eW1, eW2, eW3, eW4, eW5,
                 s_seq, s_pair, e_seq, e_pair, cross_w):
    s_pair = np.asarray(s_pair, np.float32)
    e_pair = np.asarray(e_pair, np.float32)
    s_seq = np.asarray(s_seq, np.float32)
    e_seq = np.asarray(e_seq, np.float32)
    cross_w = np.asarray(cross_w, np.float32)
    assert np.allclose(s_pair, e_pair) and np.allclose(s_seq, e_seq)
    assert np.allclose(s_pair, s_pair[0]) and np.allclose(s_seq, s_seq[0])
    pw = float(s_pair[0])
    sw = float(s_seq[0])
    # build_program bakes these as ACT drain scales
    assert pw == 0.5 and sw == 0.25, (pw, sw)
    pack = np.zeros((128, PACK_W), np.float32)
    w1s = np.asarray(sW1, np.float32).T.reshape(NCHUNK, 128, 96)
    w1e = np.asarray(eW1, np.float32).T.reshape(NCHUNK, 128, 96)
    for c in range(NCHUNK):
        pack[:, c * 96:(c + 1) * 96] = w1s[c]
        pack[:, 576 + c * 96:576 + (c + 1) * 96] = w1e[c]
    pack[0:96, 1152:1200] = np.asarray(sW2, np.float32).T
    pack[0:96, 1200:1248] = np.asarray(eW2, np.float32).T
    pack[0:48, 1248:1272] = np.asarray(sW3, np.float32).T
    pack[0:48, 1272:1296] = np.asarray(eW3, np.float32).T
    pack[0:24, 1296:1308] = np.asarray(sW4, np.float32).T
    pack[0:24, 1308:1320] = np.asarray(eW4, np.float32).T
    pack[0:12, 1320:1322] = cross_w[0] * np.asarray(sW5, np.float32).T
    pack[0:12, 1322:1324] = cross_w[1] * np.asarray(sW5, np.float32).T
    pack[0:12, 1324:1326] = np.repeat(
        np.asarray(eW5, np.float32).sum(axis=0)[:, None], 2, axis=1)
    pack[:, 1326:1454] = np.eye(128, dtype=np.float32)
    return {"wpack": pack}


def kernel(**inputs) -> np.ndarray:
    result = np.asarray(inputs["result"], np.float32)
    B = result.shape[0]
    per = B // N_CORES
    wmap = prep_weights(**{k: np.asarray(v) for k, v in inputs.items()
                           if k != "result"})
    nc = build_program(per)
    xs = result.reshape(B // 128, 128, 4, D)
    nb = per // 128
    in_maps = []
    for k in range(N_CORES):
        m = dict(wmap)
        m["x"] = np.ascontiguousarray(xs[k * nb:(k + 1) * nb])
        in_maps.append(m)
    res = run_bass_kernel_spmd(nc, in_maps, list(range(N_CORES)))
    return np.concatenate([r["out"].T for r in res.results], axis=0)



# revision 2
# speedup vs baseline: 5.8887x; 5.8887x over previous
import numpy as np

import concourse.bacc as bacc
import concourse.bass as bass
import concourse.tile as tile
from concourse import mybir

F32 = mybir.dt.float32
F32R = mybir.dt.float32r
BF16 = mybir.dt.bfloat16
RELU = mybir.ActivationFunctionType.Relu

N_CORES = 8
B_FULL = 65536
D = 768
NCHUNK = 6  # 768 / 128
PACK_W = 1830  # packed weight columns (see prep_weights)


def build_program(per_rows: int, passes=1, finalize=True, chain_dt=F32R,
                  stage_upto="full", dma_tiles=1, interleave=False,
                  xbufs=None, uvbufs=3, drain_alt=False,
                  out_big=True) -> bass.Bass:
    """One core's program: x [nb, 128, 4, 768] -> out [2, per_rows]^T.

    v2: scales folded into L5 weights (relu is positive-homogeneous),
    block-diagonal-packed L2..L5 (one PSUM bank per level, zero-padded
    lhsT so every matmul writes partition base 0).
    stage_upto: 'dve' | 'tpose' | 'full' - for HW stage isolation.
    dma_tiles: b-tiles per x DMA (1, 2 or 4).
    interleave: spread chain ops of super-tile st-1 between the b-tile
    groups of st so the PE/ACT queues always hold ready work."""
    assert per_rows % 512 == 0
    nb = per_rows // 128
    nst = nb // 4

    nc = bacc.Bacc()
    x_ext = nc.dram_tensor("x", [nb, 128, 4, D], F32, kind="ExternalInput")
    wp_ext = nc.dram_tensor("wpack", [128, PACK_W], chain_dt,
                            kind="ExternalInput")
    out_ext = nc.dram_tensor("out", [2, passes * per_rows], F32,
                             kind="ExternalOutput")

    if xbufs is None:
        xbufs = {1: 3, 2: 2, 4: 2}[dma_tiles]
    with tile.TileContext(nc) as tc:
        with (
            tc.tile_pool(name="const", bufs=1) as cpool,
            tc.tile_pool(name="x", bufs=xbufs) as xpool,
            tc.tile_pool(name="uv", bufs=uvbufs) as uvpool,
            tc.tile_pool(name="stage", bufs=2) as stpool,
            tc.tile_pool(name="chain_sb", bufs=2) as csb,
            tc.tile_pool(name="smalls", bufs=3) as smpool,
            tc.tile_pool(name="tpsum", bufs=2, space=bass.MemorySpace.PSUM) as tpsum,
            tc.tile_pool(name="cpsum", bufs=2, space=bass.MemorySpace.PSUM) as cpsum,
        ):
            # --- constants (see prep_weights for the column map) ---
            w1s_t = cpool.tile([128, NCHUNK, 96], chain_dt)
            w1e_t = cpool.tile([128, NCHUNK, 96], chain_dt)
            for c in range(NCHUNK):
                nc.sync.dma_start(w1s_t[:, c, :], wp_ext[:, c * 96:(c + 1) * 96])
                nc.sync.dma_start(w1e_t[:, c, :],
                                  wp_ext[:, 576 + c * 96:576 + (c + 1) * 96])
            w2s_t = cpool.tile([96, 96], chain_dt)
            w2e_t = cpool.tile([96, 96], chain_dt)
            w3p_t = cpool.tile([96, 120], chain_dt)
            w3q_t = cpool.tile([96, 120], chain_dt)
            w4_t = cpool.tile([120, 108], chain_dt)
            w5s_t = cpool.tile([108, 4], chain_dt)
            w5e_t = cpool.tile([108, 4], chain_dt)
            sel_t = cpool.tile([4, 2], chain_dt)
            idn_t = cpool.tile([128, 128], chain_dt)
            out_sb = (cpool.tile([2, per_rows], F32, name="out_sb")
                      if out_big and stage_upto == "full" else None)
            if stage_upto != "full":
                out0_sb = cpool.tile([2, 512], F32)
                nc.vector.memset(out0_sb[:], 0.0)
                for st0 in range(passes * nst):
                    nc.sync.dma_start(out_ext[:, st0 * 512:(st0 + 1) * 512],
                                      out0_sb[:])
            for t, r, c0, w in [
                (w2s_t[:], 96, 1152, 96), (w2e_t[:], 96, 1248, 96),
                (w3p_t[:], 96, 1344, 120), (w3q_t[:], 96, 1464, 120),
                (w4_t[:], 120, 1584, 108),
                (w5s_t[:], 108, 1692, 4), (w5e_t[:], 108, 1696, 4),
                (sel_t[:], 4, 1700, 2), (idn_t[:], 128, 1702, 128),
            ]:
                nc.sync.dma_start(t, wp_ext[0:r, c0:c0 + w])

            stages = {}

            def emit_btile(ps, st, bt4):
                if bt4 == 0:
                    # stage layout: [128 feat, 4 bt, 2 uw, 6 chunk, 128 row]
                    stages[(ps, st)] = {
                        "stage": stpool.tile([128, 4, 2, NCHUNK, 128],
                                             chain_dt, name="stage")}
                state = stages[(ps, st)]
                bt = st * 4 + bt4
                if bt4 % dma_tiles == 0:
                    xt = xpool.tile([128, dma_tiles, 4, D], F32, tag="xt")
                    state["xt"] = xt
                    if dma_tiles == 1:
                        nc.gpsimd.dma_start(xt[:, 0], x_ext[bt])
                    else:
                        nc.gpsimd.dma_start(
                            xt[:], x_ext[bt:bt + dma_tiles].rearrange(
                                "n p s d -> p n s d"))
                xt = state["xt"][:, bt4 % dma_tiles]
                # u = x0+x2, v = x1+x3, w = u+v
                uvw = uvpool.tile([128, 3, D], chain_dt)
                nc.vector.tensor_add(uvw[:, 0:2, :], xt[:, 0:2, :],
                                     xt[:, 2:4, :])
                nc.vector.tensor_add(uvw[:, 2, :], uvw[:, 0, :], uvw[:, 1, :])
                if stage_upto == "dve":
                    return
                tp = tpsum.tile([128, 2, NCHUNK, 128], chain_dt)
                for c in range(NCHUNK):
                    u_c = uvw[:, 0, c * 128:(c + 1) * 128]
                    w_c = uvw[:, 2, c * 128:(c + 1) * 128]
                    nc.tensor.matmul(tp[:, 0, c, :], u_c, idn_t[:],
                                     is_transpose=True, start=True, stop=True)
                    nc.tensor.matmul(tp[:, 1, c, :], w_c, idn_t[:],
                                     is_transpose=True, start=True, stop=True)
                # single unscaled relu drain for both u and w halves;
                # drain_alt alternates ACT/DVE per b-tile to split the load
                if drain_alt and bt4 % 2 == 1:
                    nc.vector.tensor_scalar_max(state["stage"][:, bt4],
                                                tp[:], 0.0)
                else:
                    nc.scalar.activation(state["stage"][:, bt4], tp[:], RELU)

            def emit_chain_part(ps, st, part):
                if stage_upto != "full":
                    if part == 3:
                        stages.pop((ps, st), None)
                    return
                state = stages[(ps, st)]
                stage = state["stage"]

                def l1_chain(uw, w1):
                    l1 = cpsum.tile([96, 512], F32, tag="c")
                    for c in range(NCHUNK):
                        nc.tensor.matmul(l1[:], w1[:, c, :],
                                         stage[:, :, uw, c, :],
                                         start=(c == 0), stop=(c == NCHUNK - 1))
                    sb = csb.tile([96, 512], chain_dt, tag="l1sb", bufs=6)
                    nc.scalar.activation(sb[:], l1[:], RELU)
                    return sb

                if part == 0:
                    state["l1"] = [l1_chain(0, w1s_t), l1_chain(0, w1e_t)]
                    return
                if part == 1:
                    state["l1"] += [l1_chain(1, w1s_t), l1_chain(1, w1e_t)]
                    return
                if part == 2:
                    ps_sb, pe_sb, ss_sb, se_sb = state["l1"]
                    # L2: per pool, s->partitions 0:48, e->48:96 of one bank
                    l2_sb = []
                    for s_in, e_in in [(ps_sb, pe_sb), (ss_sb, se_sb)]:
                        l2 = cpsum.tile([96, 512], F32, tag="c")
                        nc.tensor.matmul(l2[:], w2e_t[:], e_in[:],
                                         start=True, stop=False)
                        nc.tensor.matmul(l2[:], w2s_t[:], s_in[:],
                                         start=False, stop=True)
                        sb = csb.tile([96, 512], chain_dt, tag="l2sb", bufs=4)
                        nc.scalar.activation(sb[:], l2[:], RELU)
                        l2_sb.append(sb)
                    # L3: both pools into one bank: pair(s,e)->0:24,32:56;
                    # seq(s,e)->64:88,96:120
                    l3 = cpsum.tile([120, 512], F32, tag="c")
                    nc.tensor.matmul(l3[:], w3q_t[:], l2_sb[1][:],
                                     start=True, stop=False)
                    nc.tensor.matmul(l3[:], w3p_t[:], l2_sb[0][:],
                                     start=False, stop=True)
                    l4in = csb.tile([120, 512], chain_dt, tag="l3sb", bufs=4)
                    nc.scalar.activation(l4in[:], l3[:], RELU)
                    state["l4in"] = l4in
                    return
                # part 3: L4, L5, cross, final
                l4in = state["l4in"]
                l4 = cpsum.tile([108, 512], F32, tag="c")
                nc.tensor.matmul(l4[:], w4_t[:], l4in[:], start=True, stop=True)
                l5in = csb.tile([108, 512], chain_dt, tag="l4sb", bufs=4)
                nc.scalar.activation(l5in[:], l4[:], RELU)
                # L5: s-products and e-sums (scales+cross_w baked into w5s)
                s_ps = cpsum.tile([4, 512], F32, tag="c")
                nc.tensor.matmul(s_ps[:], w5s_t[:], l5in[:],
                                 start=True, stop=True)
                e_ps = cpsum.tile([4, 512], F32, tag="c")
                nc.tensor.matmul(e_ps[:], w5e_t[:], l5in[:],
                                 start=True, stop=True)
                # DVE can read at most one PSUM operand: stage e via SBUF
                e_sb = smpool.tile([4, 512], chain_dt, tag="esb", bufs=3)
                nc.scalar.activation(e_sb[:], e_ps[:],
                                     mybir.ActivationFunctionType.Identity)
                prod = smpool.tile([4, 512], chain_dt, tag="prod", bufs=3)
                nc.vector.tensor_mul(prod[:], s_ps[:], e_sb[:])
                fin = cpsum.tile([2, 512], F32, tag="c")
                nc.tensor.matmul(fin[:], sel_t[:], prod[:],
                                 start=True, stop=True)
                if out_big:
                    nc.vector.tensor_copy(out_sb[:, st * 512:st * 512 + 512],
                                          fin[:])
                else:
                    fin_sb = smpool.tile([2, 512], F32, tag="fin", bufs=3)
                    nc.vector.tensor_copy(fin_sb[:], fin[:])
                    col = (ps * nst + st) * 512
                    nc.sync.dma_start(out_ext[:, col:col + 512], fin_sb[:])
                stages.pop((ps, st))

            # 1-super-tile software pipeline
            for ps in range(passes):
                for st in range(nst + 1):
                    if interleave:
                        for bt4 in range(4):
                            if st < nst:
                                emit_btile(ps, st, bt4)
                            if st >= 1:
                                emit_chain_part(ps, st - 1, bt4)
                    else:
                        if st < nst:
                            for bt4 in range(4):
                                emit_btile(ps, st, bt4)
                        if st >= 1:
                            for part in range(4):
                                emit_chain_part(ps, st - 1, part)
                if out_big:
                    nc.sync.dma_start(
                        out_ext[:, ps * per_rows:(ps + 1) * per_rows],
                        out_sb[:])

    if finalize:
        nc.finalize()
    return nc


def prep_weights(sW1, sW2, sW3, sW4, sW5, eW1, eW2, eW3, eW4, eW5,
                 s_seq, s_pair, e_seq, e_pair, cross_w, dtype=np.float32):
    s_pair = np.asarray(s_pair, np.float32)
    e_pair = np.asarray(e_pair, np.float32)
    s_seq = np.asarray(s_seq, np.float32)
    e_seq = np.asarray(e_seq, np.float32)
    cross_w = np.asarray(cross_w, np.float32)
    # pooling must be identical across s/e branches and uniform so the
    # pools reduce to scaled u = x0+x2 and w = x0+x1+x2+x3
    assert np.allclose(s_pair, e_pair) and np.allclose(s_seq, e_seq)
    assert np.allclose(s_pair, s_pair[0]) and np.allclose(s_seq, s_seq[0])
    pw = float(s_pair[0])
    sw = float(s_seq[0])
    assert pw > 0 and sw > 0  # relu positive-homogeneity lets us fold
    pack = np.zeros((128, PACK_W), np.float32)
    w1s = np.asarray(sW1, np.float32).T.reshape(NCHUNK, 128, 96)
    w1e = np.asarray(eW1, np.float32).T.reshape(NCHUNK, 128, 96)
    for c in range(NCHUNK):
        pack[:, c * 96:(c + 1) * 96] = w1s[c]
        pack[:, 576 + c * 96:576 + (c + 1) * 96] = w1e[c]
    # L2 lhsT [96,96]: s in cols 0:48, e in cols 48:96 (separate tiles)
    pack[0:96, 1152:1200] = np.asarray(sW2, np.float32).T
    pack[0:96, 1296:1344] = np.asarray(eW2, np.float32).T
    # L3 pair lhsT [96,120]: rows 0:48 w3s -> cols 0:24; rows 48:96
    # w3e -> cols 32:56. seq lhsT: same weights -> cols 64:88, 96:120.
    pack[0:48, 1344:1368] = np.asarray(sW3, np.float32).T
    pack[48:96, 1376:1400] = np.asarray(eW3, np.float32).T
    pack[0:48, 1464 + 64:1464 + 88] = np.asarray(sW3, np.float32).T
    pack[48:96, 1464 + 96:1464 + 120] = np.asarray(eW3, np.float32).T
    # L4 lhsT [120,108]: (0:24->0:12 s), (32:56->32:44 e),
    # (64:88->64:76 s), (96:120->96:108 e)
    pack[0:24, 1584:1596] = np.asarray(sW4, np.float32).T
    pack[32:56, 1584 + 32:1584 + 44] = np.asarray(eW4, np.float32).T
    pack[64:88, 1584 + 64:1584 + 76] = np.asarray(sW4, np.float32).T
    pack[96:120, 1584 + 96:1584 + 108] = np.asarray(eW4, np.float32).T
    # L5 s lhsT [108,4]: pair_s rows 0:12 -> cols 0:2 scaled cw0*pw^2;
    # seq_s rows 64:76 -> cols 2:4 scaled cw1*sw^2
    pack[0:12, 1692:1694] = cross_w[0] * pw * pw * np.asarray(
        sW5, np.float32).T
    pack[64:76, 1694:1696] = cross_w[1] * sw * sw * np.asarray(
        sW5, np.float32).T
    # L5 e lhsT [108,4]: e-sums replicated into 2 cols at rows 32:44, 96:108
    esum = np.asarray(eW5, np.float32).sum(axis=0)[:, None]
    pack[32:44, 1696:1698] = np.repeat(esum, 2, axis=1)
    pack[96:108, 1698:1700] = np.repeat(esum, 2, axis=1)
    # sel [4,2]: out = prod[0:2] + prod[2:4]
    pack[0:4, 1700:1702] = np.array([[1, 0], [0, 1], [1, 0], [0, 1]],
                                    np.float32)
    pack[:, 1702:1830] = np.eye(128, dtype=np.float32)
    return {"wpack": pack.astype(dtype)}


def kernel(**inputs) -> np.ndarray:
    from concourse.bass_utils import run_bass_kernel_spmd
    result = np.asarray(inputs["result"], np.float32)
    B = result.shape[0]
    per = B // N_CORES
    wmap = prep_weights(**{k: np.asarray(v) for k, v in inputs.items()
                           if k != "result"})
    nc = build_program(per)
    xs = result.reshape(B // 128, 128, 4, D)
    nb = per // 128
    in_maps = []
    for k in range(N_CORES):
        m = dict(wmap)
        m["x"] = np.ascontiguousarray(xs[k * nb:(k + 1) * nb])
        in_maps.append(m)
    res = run_bass_kernel_spmd(nc, in_maps, list(range(N_CORES)))
    return np.concatenate([r["out"].T for r in res.results], axis=0)


# revision 3
# speedup vs baseline: 6.8765x; 1.1678x over previous
import numpy as np

import concourse.bacc as bacc
import concourse.bass as bass
import concourse.tile as tile
from concourse import mybir

F32 = mybir.dt.float32
F32R = mybir.dt.float32r
BF16 = mybir.dt.bfloat16
RELU = mybir.ActivationFunctionType.Relu

N_CORES = 8
B_FULL = 65536
D = 768
NCHUNK = 6  # 768 / 128
PACK_W = 1830  # packed weight columns (see prep_weights)


def build_program(per_rows: int, passes=1, finalize=True, chain_dt=F32R,
                  stage_upto="full", dma_tiles=1, interleave=False,
                  xbufs=None, uvbufs=3, drain_alt=False,
                  out_big=True, xq2=True) -> bass.Bass:
    """One core's program: x [nb, 128, 4, 768] -> out [2, per_rows]^T.

    v2: scales folded into L5 weights (relu is positive-homogeneous),
    block-diagonal-packed L2..L5 (one PSUM bank per level, zero-padded
    lhsT so every matmul writes partition base 0).
    stage_upto: 'dve' | 'tpose' | 'full' - for HW stage isolation.
    dma_tiles: b-tiles per x DMA (1, 2 or 4).
    interleave: spread chain ops of super-tile st-1 between the b-tile
    groups of st so the PE/ACT queues always hold ready work."""
    assert per_rows % 512 == 0
    nb = per_rows // 128
    nst = nb // 4

    nc = bacc.Bacc()
    x_ext = nc.dram_tensor("x", [nb, 128, 4, D], F32, kind="ExternalInput")
    wp_ext = nc.dram_tensor("wpack", [128, PACK_W], chain_dt,
                            kind="ExternalInput")
    out_ext = nc.dram_tensor("out", [2, passes * per_rows], F32,
                             kind="ExternalOutput")

    if xbufs is None:
        xbufs = {1: 3, 2: 2, 4: 2}[dma_tiles]
    with tile.TileContext(nc) as tc:
        with (
            tc.tile_pool(name="const", bufs=1) as cpool,
            tc.tile_pool(name="x", bufs=xbufs) as xpool,
            tc.tile_pool(name="uv", bufs=uvbufs) as uvpool,
            tc.tile_pool(name="stage", bufs=2) as stpool,
            tc.tile_pool(name="chain_sb", bufs=2) as csb,
            tc.tile_pool(name="smalls", bufs=3) as smpool,
            tc.tile_pool(name="tpsum", bufs=2, space=bass.MemorySpace.PSUM) as tpsum,
            tc.tile_pool(name="cpsum", bufs=2, space=bass.MemorySpace.PSUM) as cpsum,
        ):
            # --- constants (see prep_weights for the column map) ---
            w1s_t = cpool.tile([128, NCHUNK, 96], chain_dt)
            w1e_t = cpool.tile([128, NCHUNK, 96], chain_dt)
            for c in range(NCHUNK):
                nc.sync.dma_start(w1s_t[:, c, :], wp_ext[:, c * 96:(c + 1) * 96])
                nc.sync.dma_start(w1e_t[:, c, :],
                                  wp_ext[:, 576 + c * 96:576 + (c + 1) * 96])
            w2s_t = cpool.tile([96, 96], chain_dt)
            w2e_t = cpool.tile([96, 96], chain_dt)
            w3p_t = cpool.tile([96, 120], chain_dt)
            w3q_t = cpool.tile([96, 120], chain_dt)
            w4_t = cpool.tile([120, 108], chain_dt)
            w5s_t = cpool.tile([108, 4], chain_dt)
            w5e_t = cpool.tile([108, 4], chain_dt)
            sel_t = cpool.tile([4, 2], chain_dt)
            idn_t = cpool.tile([128, 128], chain_dt)
            out_sb = (cpool.tile([2, per_rows], F32, name="out_sb")
                      if out_big and stage_upto == "full" else None)
            if stage_upto != "full":
                out0_sb = cpool.tile([2, 512], F32)
                nc.vector.memset(out0_sb[:], 0.0)
                for st0 in range(passes * nst):
                    nc.sync.dma_start(out_ext[:, st0 * 512:(st0 + 1) * 512],
                                      out0_sb[:])
            for t, r, c0, w in [
                (w2s_t[:], 96, 1152, 96), (w2e_t[:], 96, 1248, 96),
                (w3p_t[:], 96, 1344, 120), (w3q_t[:], 96, 1464, 120),
                (w4_t[:], 120, 1584, 108),
                (w5s_t[:], 108, 1692, 4), (w5e_t[:], 108, 1696, 4),
                (sel_t[:], 4, 1700, 2), (idn_t[:], 128, 1702, 128),
            ]:
                nc.sync.dma_start(t, wp_ext[0:r, c0:c0 + w])

            stages = {}

            def emit_btile(ps, st, bt4):
                if bt4 == 0:
                    # stage layout: [128 feat, 4 bt, 2 uw, 6 chunk, 128 row]
                    stages[(ps, st)] = {
                        "stage": stpool.tile([128, 4, 2, NCHUNK, 128],
                                             chain_dt, name="stage")}
                state = stages[(ps, st)]
                bt = st * 4 + bt4
                if bt4 % dma_tiles == 0:
                    xt = xpool.tile([128, dma_tiles, 4, D], F32, tag="xt")
                    state["xt"] = xt
                    xeng = (nc.sync if xq2 and bt % 2 else nc.gpsimd)
                    if dma_tiles == 1:
                        xeng.dma_start(xt[:, 0], x_ext[bt])
                    else:
                        nc.gpsimd.dma_start(
                            xt[:], x_ext[bt:bt + dma_tiles].rearrange(
                                "n p s d -> p n s d"))
                xt = state["xt"][:, bt4 % dma_tiles]
                # u = x0+x2, v = x1+x3, w = u+v
                uvw = uvpool.tile([128, 3, D], chain_dt)
                nc.vector.tensor_add(uvw[:, 0:2, :], xt[:, 0:2, :],
                                     xt[:, 2:4, :])
                nc.vector.tensor_add(uvw[:, 2, :], uvw[:, 0, :], uvw[:, 1, :])
                if stage_upto == "dve":
                    return
                tp = tpsum.tile([128, 2, NCHUNK, 128], chain_dt)
                for c in range(NCHUNK):
                    u_c = uvw[:, 0, c * 128:(c + 1) * 128]
                    w_c = uvw[:, 2, c * 128:(c + 1) * 128]
                    nc.tensor.matmul(tp[:, 0, c, :], u_c, idn_t[:],
                                     is_transpose=True, start=True, stop=True)
                    nc.tensor.matmul(tp[:, 1, c, :], w_c, idn_t[:],
                                     is_transpose=True, start=True, stop=True)
                # single unscaled relu drain for both u and w halves;
                # drain_alt alternates ACT/DVE per b-tile to split the load
                if drain_alt and bt4 % 2 == 1:
                    nc.vector.tensor_scalar_max(state["stage"][:, bt4],
                                                tp[:], 0.0)
                else:
                    nc.scalar.activation(state["stage"][:, bt4], tp[:], RELU)

            def emit_chain_part(ps, st, part):
                if stage_upto != "full":
                    if part == 3:
                        stages.pop((ps, st), None)
                    return
                state = stages[(ps, st)]
                stage = state["stage"]

                def l1_chain(uw, w1):
                    l1 = cpsum.tile([96, 512], F32, tag="c")
                    for c in range(NCHUNK):
                        nc.tensor.matmul(l1[:], w1[:, c, :],
                                         stage[:, :, uw, c, :],
                                         start=(c == 0), stop=(c == NCHUNK - 1))
                    sb = csb.tile([96, 512], chain_dt, tag="l1sb", bufs=6)
                    nc.scalar.activation(sb[:], l1[:], RELU)
                    return sb

                if part == 0:
                    state["l1"] = [l1_chain(0, w1s_t), l1_chain(0, w1e_t)]
                    return
                if part == 1:
                    state["l1"] += [l1_chain(1, w1s_t), l1_chain(1, w1e_t)]
                    return
                if part == 2:
                    ps_sb, pe_sb, ss_sb, se_sb = state["l1"]
                    # L2: per pool, s->partitions 0:48, e->48:96 of one bank
                    l2_sb = []
                    for s_in, e_in in [(ps_sb, pe_sb), (ss_sb, se_sb)]:
                        l2 = cpsum.tile([96, 512], F32, tag="c")
                        nc.tensor.matmul(l2[:], w2e_t[:], e_in[:],
                                         start=True, stop=False)
                        nc.tensor.matmul(l2[:], w2s_t[:], s_in[:],
                                         start=False, stop=True)
                        sb = csb.tile([96, 512], chain_dt, tag="l2sb", bufs=4)
                        nc.scalar.activation(sb[:], l2[:], RELU)
                        l2_sb.append(sb)
                    # L3: both pools into one bank: pair(s,e)->0:24,32:56;
                    # seq(s,e)->64:88,96:120
                    l3 = cpsum.tile([120, 512], F32, tag="c")
                    nc.tensor.matmul(l3[:], w3q_t[:], l2_sb[1][:],
                                     start=True, stop=False)
                    nc.tensor.matmul(l3[:], w3p_t[:], l2_sb[0][:],
                                     start=False, stop=True)
                    l4in = csb.tile([120, 512], chain_dt, tag="l3sb", bufs=4)
                    nc.scalar.activation(l4in[:], l3[:], RELU)
                    state["l4in"] = l4in
                    return
                # part 3: L4, L5, cross, final
                l4in = state["l4in"]
                l4 = cpsum.tile([108, 512], F32, tag="c")
                nc.tensor.matmul(l4[:], w4_t[:], l4in[:], start=True, stop=True)
                l5in = csb.tile([108, 512], chain_dt, tag="l4sb", bufs=4)
                nc.scalar.activation(l5in[:], l4[:], RELU)
                # L5: s-products and e-sums (scales+cross_w baked into w5s)
                s_ps = cpsum.tile([4, 512], F32, tag="c")
                nc.tensor.matmul(s_ps[:], w5s_t[:], l5in[:],
                                 start=True, stop=True)
                e_ps = cpsum.tile([4, 512], F32, tag="c")
                nc.tensor.matmul(e_ps[:], w5e_t[:], l5in[:],
                                 start=True, stop=True)
                # DVE can read at most one PSUM operand: stage e via SBUF
                e_sb = smpool.tile([4, 512], chain_dt, tag="esb", bufs=3)
                nc.scalar.activation(e_sb[:], e_ps[:],
                                     mybir.ActivationFunctionType.Identity)
                prod = smpool.tile([4, 512], chain_dt, tag="prod", bufs=3)
                nc.vector.tensor_mul(prod[:], s_ps[:], e_sb[:])
                fin = cpsum.tile([2, 512], F32, tag="c")
                nc.tensor.matmul(fin[:], sel_t[:], prod[:],
                                 start=True, stop=True)
                if out_big:
                    nc.vector.tensor_copy(out_sb[:, st * 512:st * 512 + 512],
                                          fin[:])
                else:
                    fin_sb = smpool.tile([2, 512], F32, tag="fin", bufs=3)
                    nc.vector.tensor_copy(fin_sb[:], fin[:])
                    col = (ps * nst + st) * 512
                    nc.sync.dma_start(out_ext[:, col:col + 512], fin_sb[:])
                stages.pop((ps, st))

            # 1-super-tile software pipeline
            for ps in range(passes):
                for st in range(nst + 1):
                    if interleave:
                        for bt4 in range(4):
                            if st < nst:
                                emit_btile(ps, st, bt4)
                            if st >= 1:
                                emit_chain_part(ps, st - 1, bt4)
                    else:
                        if st < nst:
                            for bt4 in range(4):
                                emit_btile(ps, st, bt4)
                        if st >= 1:
                            for part in range(4):
                                emit_chain_part(ps, st - 1, part)
                if out_big:
                    nc.sync.dma_start(
                        out_ext[:, ps * per_rows:(ps + 1) * per_rows],
                        out_sb[:])

    if finalize:
        nc.finalize()
    return nc


def prep_weights(sW1, sW2, sW3, sW4, sW5, eW1, eW2, eW3, eW4, eW5,
                 s_seq, s_pair, e_seq, e_pair, cross_w, dtype=np.float32):
    s_pair = np.asarray(s_pair, np.float32)
    e_pair = np.asarray(e_pair, np.float32)
    s_seq = np.asarray(s_seq, np.float32)
    e_seq = np.asarray(e_seq, np.float32)
    cross_w = np.asarray(cross_w, np.float32)
    # pooling must be identical across s/e branches and uniform so the
    # pools reduce to scaled u = x0+x2 and w = x0+x1+x2+x3
    assert np.allclose(s_pair, e_pair) and np.allclose(s_seq, e_seq)
    assert np.allclose(s_pair, s_pair[0]) and np.allclose(s_seq, s_seq[0])
    pw = float(s_pair[0])
    sw = float(s_seq[0])
    assert pw > 0 and sw > 0  # relu positive-homogeneity lets us fold
    pack = np.zeros((128, PACK_W), np.float32)
    w1s = np.asarray(sW1, np.float32).T.reshape(NCHUNK, 128, 96)
    w1e = np.asarray(eW1, np.float32).T.reshape(NCHUNK, 128, 96)
    for c in range(NCHUNK):
        pack[:, c * 96:(c + 1) * 96] = w1s[c]
        pack[:, 576 + c * 96:576 + (c + 1) * 96] = w1e[c]
    # L2 lhsT [96,96]: s in cols 0:48, e in cols 48:96 (separate tiles)
    pack[0:96, 1152:1200] = np.asarray(sW2, np.float32).T
    pack[0:96, 1296:1344] = np.asarray(eW2, np.float32).T
    # L3 pair lhsT [96,120]: rows 0:48 w3s -> cols 0:24; rows 48:96
    # w3e -> cols 32:56. seq lhsT: same weights -> cols 64:88, 96:120.
    pack[0:48, 1344:1368] = np.asarray(sW3, np.float32).T
    pack[48:96, 1376:1400] = np.asarray(eW3, np.float32).T
    pack[0:48, 1464 + 64:1464 + 88] = np.asarray(sW3, np.float32).T
    pack[48:96, 1464 + 96:1464 + 120] = np.asarray(eW3, np.float32).T
    # L4 lhsT [120,108]: (0:24->0:12 s), (32:56->32:44 e),
    # (64:88->64:76 s), (96:120->96:108 e)
    pack[0:24, 1584:1596] = np.asarray(sW4, np.float32).T
    pack[32:56, 1584 + 32:1584 + 44] = np.asarray(eW4, np.float32).T
    pack[64:88, 1584 + 64:1584 + 76] = np.asarray(sW4, np.float32).T
    pack[96:120, 1584 + 96:1584 + 108] = np.asarray(eW4, np.float32).T
    # L5 s lhsT [108,4]: pair_s rows 0:12 -> cols 0:2 scaled cw0*pw^2;
    # seq_s rows 64:76 -> cols 2:4 scaled cw1*sw^2
    pack[0:12, 1692:1694] = cross_w[0] * pw * pw * np.asarray(
        sW5, np.float32).T
    pack[64:76, 1694:1696] = cross_w[1] * sw * sw * np.asarray(
        sW5, np.float32).T
    # L5 e lhsT [108,4]: e-sums replicated into 2 cols at rows 32:44, 96:108
    esum = np.asarray(eW5, np.float32).sum(axis=0)[:, None]
    pack[32:44, 1696:1698] = np.repeat(esum, 2, axis=1)
    pack[96:108, 1698:1700] = np.repeat(esum, 2, axis=1)
    # sel [4,2]: out = prod[0:2] + prod[2:4]
    pack[0:4, 1700:1702] = np.array([[1, 0], [0, 1], [1, 0], [0, 1]],
                                    np.float32)
    pack[:, 1702:1830] = np.eye(128, dtype=np.float32)
    return {"wpack": pack.astype(dtype)}


def kernel(**inputs) -> np.ndarray:
    from concourse.bass_utils import run_bass_kernel_spmd
    result = np.asarray(inputs["result"], np.float32)
    B = result.shape[0]
    per = B // N_CORES
    wmap = prep_weights(**{k: np.asarray(v) for k, v in inputs.items()
                           if k != "result"})
    nc = build_program(per)
    xs = result.reshape(B // 128, 128, 4, D)
    nb = per // 128
    in_maps = []
    for k in range(N_CORES):
        m = dict(wmap)
        m["x"] = np.ascontiguousarray(xs[k * nb:(k + 1) * nb])
        in_maps.append(m)
    res = run_bass_kernel_spmd(nc, in_maps, list(range(N_CORES)))
    return np.concatenate([r["out"].T for r in res.results], axis=0)
